# revision 1
# baseline (speedup 1.0000x reference)
"""Trainium2 Bass kernel for nn_CodecAttention (sliding-window ALiBi attention).

Reference computation (B=4, T=2048, DIM=1024, H=8, HD=128, WINDOW=16):
    xq = rms_norm(x @ wq) ; xk = rms_norm(x @ wk) ; xv = x @ wv
    scores = q k^T / sqrt(HD) + alibi_bias  (causal + 16-token sliding window)
    out = softmax(scores) @ v  -> reshape -> @ wo

Sharding: 8 cores = (batch b, sequence half). Each core processes 1024 query
tokens plus a 128-token key/value halo (zeros for the first half), fully
locally -- the attention window (16) never crosses the halo, so no
collectives are needed.

Layout strategy (per core): everything transposed. Host passes xT [DIM, 1152].
Projections produce qT/kT in [dim, tok] layout and v in natural [tok, dim]
layout. Scores are computed transposed (sT[k, q] = kT.T @ qT per head), the
softmax denominator comes from a ones-column matmul (reduction over the
partition axis), and PV produces attn_outT[d, q] = v.T-free matmul with
exp(sT) as the moving operand. attn_outT is exactly the stationary operand the
wo matmul wants, so the final output lands in natural [tok, dim] layout with
zero transposes anywhere.

All matmuls run in float32r (full PE rate at N>=256, ~1.6e-4 rel err/K=128).
RMS norm: sum-of-squares via ones-matmul, rsqrt via ACT Sqrt + DVE reciprocal,
applied through a K=1 broadcast matmul (rstd per token broadcast across
partitions; the k-side broadcast uses q_norm_w*k_norm_w/sqrt(HD) as the
stationary operand, folding the norm weights and score scale in for free).

ALiBi + causal + window mask: tiny per-(j) rel tiles with -1e9 at invalid
positions; scores += slope_h * rel via one fused scalar_tensor_tensor. The
first key tile of the first q-chunk additionally subtracts a per-core
"negcol" column that kills out-of-range (global position < 0) halo keys.
"""

import math
import os

import numpy as np

os.environ.setdefault("MYCRO_LOCAL_CACHE", "1")

import concourse.mybir as mybir
import concourse.tile as tile
from concourse import bacc
from concourse.bass_utils import run_bass_kernel_spmd

F32 = mybir.dt.float32
F32R = mybir.dt.float32r
AF = mybir.ActivationFunctionType
ALU = mybir.AluOpType

B, T, DIM = 4, 2048, 1024
H, HD = 8, 128
WINDOW = 16
EPS = 1e-6
NEG = -1.0e9
BIGMASK = 30000.0

HALO = 128                 # key/value halo tokens per shard
TSH = HALO + T // 2        # 1152 tokens per shard
QTOK = T // 2              # 1024 query tokens per shard
ND = DIM // 128            # 8 dim tiles
NT = TSH // 128            # 9 token tiles
QC = 256                   # attention query-chunk width
NQC = QTOK // QC           # 4 query chunks
K_CHUNKS = [(0, 384), (384, 384), (768, 384)]        # kT projection chunks
Q_CHUNKS = [(0, 512), (512, 512)]                    # qT projection chunks

_SLOPES = [2.0 ** (-i) for i in range(H)]

_CACHE = {}


def _build_program():
    nc = bacc.Bacc("TRN2", debug=False, target_bir_lowering=False, num_devices=8)

    xt = nc.declare_dram_parameter("xt", [128, ND, TSH], F32R, isOutput=False)
    wq = nc.declare_dram_parameter("wq", [DIM, DIM], F32R, isOutput=False)
    wk = nc.declare_dram_parameter("wk", [DIM, DIM], F32R, isOutput=False)
    wv = nc.declare_dram_parameter("wv", [DIM, DIM], F32R, isOutput=False)
    wo = nc.declare_dram_parameter("wo", [DIM, DIM], F32R, isOutput=False)
    qkw_row = nc.declare_dram_parameter("qkw_row", [1, ND, 128], F32R, isOutput=False)
    ones_row = nc.declare_dram_parameter("ones_row", [1, 128], F32R, isOutput=False)
    ones_col = nc.declare_dram_parameter("ones_col", [128, 1], F32R, isOutput=False)
    rel4 = nc.declare_dram_parameter("rel4", [128, 4, QC], F32, isOutput=False)
    out = nc.declare_dram_parameter("out", [QTOK, DIM], F32, isOutput=True)

    with tile.TileContext(nc) as tc:
        with tc.tile_pool(name="big", bufs=1) as big:
            # ---- constants + persistent tensors (live for the whole kernel) ----
            kt_sb = big.tile([128, ND, TSH], F32R)
            qt_sb = big.tile([128, ND, QTOK], F32R)
            v_sb = big.tile([128, NT, DIM], F32R)
            qkw_sb = big.tile([1, ND, 128], F32R)
            onesr_sb = big.tile([1, 128], F32R)
            onesc_sb = big.tile([128, 1], F32R)
            rel4_sb = big.tile([128, 4, QC], F32)
            eps_sb = big.tile([1, 1], F32)
            nc.vector.memset(eps_sb[:], EPS)
            nc.sync.dma_start(qkw_sb[:], qkw_row[:])
            nc.sync.dma_start(onesr_sb[:], ones_row[:])
            nc.sync.dma_start(onesc_sb[:], ones_col[:])
            nc.sync.dma_start(rel4_sb[:], rel4[:])

            self_phase1(tc, nc, kt_sb, qt_sb, v_sb, qkw_sb, onesr_sb, onesc_sb,
                        eps_sb, xt, wq, wk, wv)
            self_phase2(tc, nc, kt_sb, qt_sb, v_sb, onesr_sb, onesc_sb,
                        rel4_sb, wo, out)
    nc.compile()
    return nc


def self_phase1(tc, nc, kt_sb, qt_sb, v_sb, qkw_sb, onesr_sb, onesc_sb,
                eps_sb, xt, wq, wk, wv):
    with (
        tc.tile_pool(name="xtp", bufs=1) as xtp,
        tc.tile_pool(name="wp", bufs=int(os.environ.get("KP_WP", 10))) as wp,
        tc.tile_pool(name="scr", bufs=2) as scrp,
        tc.tile_pool(name="sqt", bufs=1) as sqtp,
        tc.tile_pool(name="rcp", bufs=2) as rcpp,
        tc.tile_pool(name="pp", bufs=int(os.environ.get("KP_PP", 6)),
                     space="PSUM") as pp,
        tc.tile_pool(name="sqp", bufs=int(os.environ.get("KP_SQP", 1)),
                     space="PSUM") as sqp,
        tc.tile_pool(name="bcp", bufs=int(os.environ.get("KP_BCP", 1)),
                     space="PSUM") as bcp,
    ):
            xt_sb = xtp.tile([128, ND, TSH], F32R)

            # ---- projections: kT and qT (with RMS-norm), v (plain) ----
            def drain_ps(dst, ps, m, c0, cw, ssq):
                # raw copy (rounded to f32r) + square + ssq accumulation;
                # alternate engines to balance ACT vs DVE load
                if m % 2 == 0:
                    nc.scalar.copy(dst[:, m, c0:c0 + cw], ps[:, :cw])
                else:
                    nc.vector.tensor_copy(dst[:, m, c0:c0 + cw], ps[:, :cw])
                sq = scrp.tile([128, 512], F32R, tag="sq")
                if m % 2 == 0:
                    # DVE square must read the SBUF copy (one-PSUM-input rule)
                    nc.vector.tensor_mul(sq[:, :cw], dst[:, m, c0:c0 + cw],
                                         dst[:, m, c0:c0 + cw])
                else:
                    nc.scalar.square(sq[:, :cw], ps[:, :cw])
                nc.tensor.matmul(
                    ssq[:, :cw], onesc_sb[:], sq[:, :cw],
                    start=(m == 0), stop=(m == ND - 1),
                )

            def proj_normed(w_dram, dst, chunks, tok0, fold_qkw, first=False,
                            pool=None):
                pool = pool or pp
                """dst[:, m, c] = rstd * (x @ w)^T, rstd from raw sum-of-squares."""
                w_slices = []
                for kk in range(ND):
                    w_sl = wp.tile([128, DIM], F32R, tag="wslice")
                    nc.sync.dma_start(w_sl[:], w_dram[kk * 128:(kk + 1) * 128, :])
                    if first:
                        # interleave xt loads so the kk-outer first chunk can
                        # start as soon as the first (w, xt) slice pair lands
                        nc.sync.dma_start(xt_sb[:, kk, :], xt[:, kk, :])
                    w_slices.append(w_sl)
                for ci, (c0, cw) in enumerate(chunks):
                    ssq = sqp.tile([1, 512], F32)
                    if first and ci == 0:
                        # kk-outer in m-blocks of 4: PE consumes DMA'd slices
                        # incrementally instead of waiting for all 16
                        for mb in range(0, ND, 4):
                            blk = []
                            for m in range(mb, mb + 4):
                                ps = pool.tile([128, 512], F32, tag="ps")
                                blk.append(ps)
                            for kk in range(ND):
                                for mi, m in enumerate(range(mb, mb + 4)):
                                    nc.tensor.matmul(
                                        blk[mi][:, :cw],
                                        w_slices[kk][:, m * 128:(m + 1) * 128],
                                        xt_sb[:, kk, tok0 + c0: tok0 + c0 + cw],
                                        start=(kk == 0), stop=(kk == ND - 1),
                                    )
                            for mi, m in enumerate(range(mb, mb + 4)):
                                drain_ps(dst, blk[mi], m, c0, cw, ssq)
                    else:
                        for m in range(ND):
                            ps = pool.tile([128, 512], F32, tag="ps")
                            for kk in range(ND):
                                nc.tensor.matmul(
                                    ps[:, :cw],
                                    w_slices[kk][:, m * 128:(m + 1) * 128],
                                    xt_sb[:, kk, tok0 + c0: tok0 + c0 + cw],
                                    start=(kk == 0), stop=(kk == ND - 1),
                                )
                            drain_ps(dst, ps, m, c0, cw, ssq)
                    sqt = sqtp.tile([1, 512], F32, tag="sqt")
                    nc.scalar.activation(sqt[:, :cw], ssq[:, :cw], AF.Sqrt,
                                         bias=eps_sb[:], scale=1.0 / DIM)
                    rstd = rcpp.tile([1, 512], F32R, tag="rstd")
                    with nc.allow_low_precision(reason="f32r rstd for matmul"):
                        nc.vector.reciprocal(rstd[:, :cw], sqt[:, :cw])
                    if fold_qkw:
                        for m in range(ND):
                            rsb = bcp.tile([128, 512], F32)
                            nc.tensor.matmul(rsb[:, :cw], qkw_sb[:, m, :],
                                             rstd[:, :cw], start=True, stop=True)
                            nc.vector.tensor_mul(dst[:, m, c0:c0 + cw],
                                                 dst[:, m, c0:c0 + cw], rsb[:, :cw])
                    else:
                        rsb = bcp.tile([128, 512], F32)
                        nc.tensor.matmul(rsb[:, :cw], onesr_sb[:],
                                         rstd[:, :cw], start=True, stop=True)
                        # stage the broadcast in SBUF: frees the psum slot and
                        # keeps the 8 muls off the one-PSUM-operand path
                        rsb_sb = scrp.tile([128, 512], F32, tag="rsbsb")
                        nc.scalar.copy(rsb_sb[:, :cw], rsb[:, :cw])
                        for m in range(ND):
                            nc.vector.tensor_mul(dst[:, m, c0:c0 + cw],
                                                 dst[:, m, c0:c0 + cw],
                                                 rsb_sb[:, :cw])

            proj_normed(wk, kt_sb, K_CHUNKS, 0, fold_qkw=True, first=True)
            proj_normed(wq, qt_sb, Q_CHUNKS, HALO, fold_qkw=False)

            # v: natural layout [tok, dim]
            wv_slices = []
            for kk in range(ND):
                w_sl = wp.tile([128, DIM], F32R, tag="wslice")
                nc.sync.dma_start(w_sl[:], wv[kk * 128:(kk + 1) * 128, :])
                wv_slices.append(w_sl)
            for tt in range(NT):
                for nn in range(2):
                    ps = pp.tile([128, 512], F32)
                    for kk in range(ND):
                        nc.tensor.matmul(
                            ps[:],
                            xt_sb[:, kk, tt * 128:(tt + 1) * 128],
                            wv_slices[kk][:, nn * 512:(nn + 1) * 512],
                            start=(kk == 0), stop=(kk == ND - 1),
                        )
                    if tt % 2 == 0:
                        nc.scalar.copy(v_sb[:, tt, nn * 512:(nn + 1) * 512], ps[:])
                    else:
                        nc.vector.tensor_copy(v_sb[:, tt, nn * 512:(nn + 1) * 512],
                                              ps[:])


def self_phase2(tc, nc, kt_sb, qt_sb, v_sb, onesr_sb, onesc_sb,
                rel4_sb, wo, out):
        # xt freed; load wo and run attention + output projection
        with (
            tc.tile_pool(name="wo", bufs=1) as wop,
            tc.tile_pool(name="exp", bufs=int(os.environ.get("KP_EXP", 3))) as expp,
            tc.tile_pool(name="atc", bufs=int(os.environ.get("KP_ATC", 2))) as atcp,
            tc.tile_pool(name="outp", bufs=3) as outp,
            tc.tile_pool(name="rcp2", bufs=2) as rcp2p,
            tc.tile_pool(name="sps", bufs=int(os.environ.get("KP_SPS", 2)),
                         space="PSUM") as sps,
            tc.tile_pool(name="ytp", bufs=int(os.environ.get("KP_YTP", 1)),
                         space="PSUM") as ytp,
            tc.tile_pool(name="rsp", bufs=int(os.environ.get("KP_RSP", 1)),
                         space="PSUM") as rsp,
            tc.tile_pool(name="bc2", bufs=int(os.environ.get("KP_BC2", 1)),
                         space="PSUM") as bc2p,
            tc.tile_pool(name="pso", bufs=int(os.environ.get("KP_PSO", 1)),
                         space="PSUM") as psop,
        ):
            wo_sb = wop.tile([128, ND, DIM], F32R)
            for hd in range(ND):
                nc.sync.dma_start(wo_sb[:, hd, :], wo[hd * 128:(hd + 1) * 128, :])

            for qc in range(NQC):
                aT = atcp.tile([128, ND, QC], F32R)
                for h in range(H):
                    yT_t = ytp.tile([128, QC], F32, tag="yT")
                    rs_t = rsp.tile([1, QC], F32, tag="rs")
                    yT = yT_t[:, :]
                    rs = rs_t[:, :]
                    # joint [128, 3, QC] score tile: three QK matmuls, then ONE
                    # fused bias-add and ONE exp over all 768 columns.
                    # rel4 slots: [0]=j0-first-tile variant (per-core: all-NEG
                    # on first-half cores), [1]=j1, [2]=j2, [3]=j0-regular.
                    # qc=0 uses rel4[0:3] with slots (j0,j1,j2); qc>0 uses
                    # rel4[1:4] with slots (j1,j2,j0).
                    jmap = (0, 1, 2) if qc == 0 else (1, 2, 0)
                    rel_w = rel4_sb[:, 0:3, :] if qc == 0 else rel4_sb[:, 1:4, :]
                    stj = sps.tile([128, 3, QC], F32)
                    st = stj[:, 0:3, :]
                    for s, j in enumerate(jmap):
                        nc.tensor.matmul(
                            stj[:, s, :],
                            kt_sb[:, h, qc * QC + j * 128: qc * QC + (j + 1) * 128],
                            qt_sb[:, h, qc * QC: (qc + 1) * QC],
                            start=True, stop=True,
                        )
                    # scores += slope_h * rel (rel = -1e9 at masked positions)
                    nc.vector.scalar_tensor_tensor(
                        out=st[:], in0=rel_w, scalar=_SLOPES[h],
                        in1=st[:], op0=ALU.mult, op1=ALU.add)
                    ex = expp.tile([128, 3, QC], F32R, tag="exp")
                    nc.scalar.activation(ex[:], st[:], AF.Exp)
                    for s, j in enumerate(jmap):
                        nc.tensor.matmul(
                            yT,
                            v_sb[:, 2 * qc + j, h * 128:(h + 1) * 128],
                            ex[:, s, :], start=(s == 0), stop=(s == 2),
                        )
                        nc.tensor.matmul(
                            rs, onesc_sb[:], ex[:, s, :],
                            start=(s == 0), stop=(s == 2),
                        )
                    rcp = rcp2p.tile([1, QC], F32R, tag="rcp")
                    with nc.allow_low_precision(reason="f32r prob scale"):
                        nc.vector.reciprocal(rcp[:], rs)
                    rsb2_t = bc2p.tile([128, QC], F32, tag="rsb2")
                    rsb2 = rsb2_t[:, :]
                    nc.tensor.matmul(rsb2, onesr_sb[:], rcp[:],
                                     start=True, stop=True)
                    nc.scalar.copy(aT[:, h, :], yT)
                    nc.vector.tensor_mul(aT[:, h, :], aT[:, h, :], rsb2)

                # output projection for this q-chunk
                for t2 in range(QC // 128):
                    for nn in range(2):
                        ps_o = psop.tile([128, 512], F32)
                        for hd in range(ND):
                            nc.tensor.matmul(
                                ps_o[:],
                                aT[:, hd, t2 * 128:(t2 + 1) * 128],
                                wo_sb[:, hd, nn * 512:(nn + 1) * 512],
                                start=(hd == 0), stop=(hd == ND - 1),
                            )
                        o_sb = outp.tile([128, 512], F32, tag="osb")
                        nc.vector.tensor_copy(o_sb[:], ps_o[:])
                        nc.sync.dma_start(
                            out[qc * QC + t2 * 128: qc * QC + (t2 + 1) * 128,
                                nn * 512:(nn + 1) * 512],
                            o_sb[:],
                        )


def _host_constants():
    # relpat(j)[kj, qi] = 128*(j-1) + kj - qi if in window else NEG
    kj = np.arange(128)[:, None, None]
    jj = np.arange(3)[None, :, None]
    qi = np.arange(QC)[None, None, :]
    rel = 128 * (jj - 1) + kj - qi
    valid = (rel <= 0) & (rel >= -WINDOW)
    relpat = np.where(valid, rel, NEG).astype(np.float32)  # [128, 3, QC]
    ones_row = np.ones((1, 128), dtype=np.float32)
    ones_col = np.ones((128, 1), dtype=np.float32)
    return relpat, ones_row, ones_col


def _make_in_maps(x, wq, wk, wv, wo, q_norm_w, k_norm_w):
    x = np.ascontiguousarray(np.asarray(x, dtype=np.float32))
    wq = np.ascontiguousarray(np.asarray(wq, dtype=np.float32))
    wk = np.ascontiguousarray(np.asarray(wk, dtype=np.float32))
    wv = np.ascontiguousarray(np.asarray(wv, dtype=np.float32))
    wo = np.ascontiguousarray(np.asarray(wo, dtype=np.float32))
    q_norm_w = np.asarray(q_norm_w, dtype=np.float32)
    k_norm_w = np.asarray(k_norm_w, dtype=np.float32)

    relpat, ones_row, ones_col = _host_constants()
    qkw = (q_norm_w * k_norm_w / math.sqrt(HD)).astype(np.float32)
    qkw_row = qkw.reshape(1, ND, 128)

    in_maps = []
    for c in range(8):
        b, hf = c // 2, c % 2
        base = hf * (T // 2)
        xsh = np.zeros((TSH, DIM), dtype=np.float32)
        lo = base - HALO
        if lo < 0:
            xsh[HALO:] = x[b, base: base + QTOK]
        else:
            xsh[:] = x[b, lo: base + QTOK]
        xt_c = np.ascontiguousarray(
            xsh.T.reshape(ND, 128, TSH).transpose(1, 0, 2))
        rel4 = np.empty((128, 4, QC), dtype=np.float32)
        rel4[:, 1:3, :] = relpat[:, 1:3, :]          # j1, j2
        rel4[:, 3, :] = relpat[:, 0, :]              # j0 regular
        rel4[:, 0, :] = NEG if hf == 0 else relpat[:, 0, :]  # j0 first tile
        in_maps.append({
            "xt": xt_c, "wq": wq, "wk": wk, "wv": wv, "wo": wo,
            "qkw_row": qkw_row, "ones_row": ones_row, "ones_col": ones_col,
            "rel4": rel4,
        })

    return in_maps


def kernel(x, wq, wk, wv, wo, q_norm_w, k_norm_w):
    if "nc" not in _CACHE:
        _CACHE["nc"] = _build_program()
    nc = _CACHE["nc"]
    in_maps = _make_in_maps(x, wq, wk, wv, wo, q_norm_w, k_norm_w)
    _CACHE["in_maps"] = in_maps
    import time as _time
    last_err = None
    for attempt in range(3):
        try:
            res = run_bass_kernel_spmd(nc, in_maps, core_ids=list(range(8)))
            break
        except Exception as e:  # transient NRT/device wedges recover on retry
            last_err = e
            _time.sleep(10 * (attempt + 1))
    else:
        raise last_err

    out = np.empty((B, T, DIM), dtype=np.float32)
    for c in range(8):
        b, hf = c // 2, c % 2
        out[b, hf * QTOK:(hf + 1) * QTOK, :] = res.results[c]["out"]
    return out



# revision 7
# speedup vs baseline: 1.0987x; 1.0987x over previous
"""Trainium2 Bass kernel for nn_CodecAttention (sliding-window ALiBi attention).

Reference computation (B=4, T=2048, DIM=1024, H=8, HD=128, WINDOW=16):
    xq = rms_norm(x @ wq) ; xk = rms_norm(x @ wk) ; xv = x @ wv
    scores = q k^T / sqrt(HD) + alibi_bias  (causal + 16-token sliding window)
    out = softmax(scores) @ v  -> reshape -> @ wo

Sharding: 8 cores = (batch b, sequence half). Each core processes 1024 query
tokens plus a 16-token key/value halo (zeros for the first half), fully
locally -- the attention window (16) never crosses the halo, so no
collectives are needed.

Banded attention: queries are processed in chunks of QW=112; each chunk's
entire window [q0-16, q0+111] fits in ONE unaligned 128-wide key tile
(SBUF free-axis slices of the transposed kT/qT are unaligned-friendly, and
v is projected directly into per-chunk banded [key, dim] tiles using
unaligned free-slices of x^T as the stationary operand). Per chunk+head:
one QK matmul, one PV matmul; the softmax denominator is produced ALREADY
BROADCAST across partitions by a single matmul with an all-ones [128,128]
stationary. ALiBi bias + causal/window mask is one tensor_add with a
host-precomputed per-4-head tile (slope folded in, -1e9 at invalid).

Projections produce qT/kT in [dim, tok] layout via f32r matmuls (full PE
rate at N>=256). RMS norm: sum-of-squares via ones-matmul over the psum
drain, rstd in ONE fused ACT Rsqrt (the uniform q_norm*k_norm/sqrt(HD)
score scale folds into the k-side Rsqrt scale/bias since rms_norm is
uniform-scale invariant), broadcast via a K=1 matmul, applied on DVE.
"""

import math
import os

import numpy as np

os.environ.setdefault("MYCRO_LOCAL_CACHE", "1")

import concourse.mybir as mybir
import concourse.tile as tile
from concourse import bacc
from concourse.bass_utils import run_bass_kernel_spmd

F32 = mybir.dt.float32
F32R = mybir.dt.float32r
AF = mybir.ActivationFunctionType
ALU = mybir.AluOpType

B, T, DIM = 4, 2048, 1024
H, HD = 8, 128
WINDOW = 16
EPS = 1e-6
NEG = -1.0e9

HALO = 16                  # key/value halo tokens per shard
TSH = HALO + T // 2        # 1040 tokens per shard
QTOK = T // 2              # 1024 query tokens per shard
ND = DIM // 128            # 8 dim tiles
QW = 112                   # attention query-chunk width (window fits 128 keys)
NCH = 10                   # 9 full chunks + 1 tail chunk (16 queries)
K_CHUNKS = [(0, 384), (384, 384), (768, 272)]        # kT projection chunks
Q_CHUNKS = [(0, 512), (512, 512)]                    # qT projection chunks

_CACHE = {}


def _chunk_geom(c):
    """(k-window col start, k width, query col start in qt, query width)."""
    if c < 9:
        return 112 * c, 128, 112 * c, 112
    return 1008, 32, 1008, 16


def _build_program(kscale, keps):
    nc = bacc.Bacc("TRN2", debug=False, target_bir_lowering=False, num_devices=8)

    xt = nc.declare_dram_parameter("xt", [128, ND, TSH], F32R, isOutput=False)
    wq = nc.declare_dram_parameter("wq", [DIM, DIM], F32R, isOutput=False)
    wk = nc.declare_dram_parameter("wk", [DIM, DIM], F32R, isOutput=False)
    wv = nc.declare_dram_parameter("wv", [DIM, DIM], F32R, isOutput=False)
    wo = nc.declare_dram_parameter("wo", [DIM, DIM], F32R, isOutput=False)
    relg = nc.declare_dram_parameter("relg", [128, 2, H, QW], F32, isOutput=False)
    reltail = nc.declare_dram_parameter("reltail", [32, H, 16], F32, isOutput=False)
    ones_sq = nc.declare_dram_parameter("ones_sq", [128, 128], F32R, isOutput=False)
    ones_row = nc.declare_dram_parameter("ones_row", [1, 128], F32R, isOutput=False)
    ones_col = nc.declare_dram_parameter("ones_col", [128, 1], F32R, isOutput=False)
    out = nc.declare_dram_parameter("out", [QTOK, DIM], F32, isOutput=True)

    with tile.TileContext(nc) as tc:
        with tc.tile_pool(name="big", bufs=1) as big:
            # ---- constants + persistent tensors (live for the whole kernel) ----
            kt_sb = big.tile([128, ND, TSH], F32R)
            qt_sb = big.tile([128, ND, QTOK], F32R)
            v_sb = big.tile([128, NCH, DIM], F32R)
            ones128 = big.tile([128, 128], F32R)
            onesr_sb = big.tile([1, 128], F32R)
            onesc_sb = big.tile([128, 1], F32R)
            relg_sb = big.tile([128, 2, H, QW], F32)
            relt_sb = big.tile([32, H, 16], F32)
            epsq_sb = big.tile([1, 1], F32)
            epsk_sb = big.tile([1, 1], F32)
            nc.vector.memset(epsq_sb[:], EPS)
            nc.vector.memset(epsk_sb[:], keps)
            nc.sync.dma_start(ones128[:], ones_sq[:])
            nc.sync.dma_start(onesr_sb[:], ones_row[:])
            nc.sync.dma_start(onesc_sb[:], ones_col[:])
            nc.sync.dma_start(relg_sb[:], relg[:])
            nc.sync.dma_start(relt_sb[:], reltail[:])

            self_phase1(tc, nc, kt_sb, qt_sb, v_sb, onesr_sb, onesc_sb,
                        epsq_sb, epsk_sb, kscale, xt, wq, wk, wv)
            self_phase2(tc, nc, kt_sb, qt_sb, v_sb, ones128,
                        relg_sb, relt_sb, wo, out)
    nc.compile()
    return nc


def self_phase1(tc, nc, kt_sb, qt_sb, v_sb, onesr_sb, onesc_sb,
                epsq_sb, epsk_sb, kscale, xt, wq, wk, wv):
    with (
        tc.tile_pool(name="xtp", bufs=1) as xtp,
        tc.tile_pool(name="wp", bufs=int(os.environ.get("KP_WP", 8))) as wp,
        tc.tile_pool(name="scr", bufs=2) as scrp,
        tc.tile_pool(name="rcp", bufs=2) as rcpp,
        tc.tile_pool(name="pp", bufs=int(os.environ.get("KP_PP", 6)),
                     space="PSUM") as pp,
        tc.tile_pool(name="sqp", bufs=int(os.environ.get("KP_SQP", 1)),
                     space="PSUM") as sqp,
        tc.tile_pool(name="bcp", bufs=int(os.environ.get("KP_BCP", 1)),
                     space="PSUM") as bcp,
    ):
            xt_sb = xtp.tile([128, ND, TSH], F32R)

            # ---- projections: kT and qT (with RMS-norm), v (banded tiles) ----
            def drain_ps(dst, ps, m, c0, cw, ssq):
                # raw copy (rounded to f32r) + square + ssq accumulation;
                # alternate engines to balance ACT vs DVE load
                if m % 2 == 0:
                    nc.scalar.copy(dst[:, m, c0:c0 + cw], ps[:, :cw])
                else:
                    nc.vector.tensor_copy(dst[:, m, c0:c0 + cw], ps[:, :cw])
                sq = scrp.tile([128, 512], F32R, tag="sq")
                if m % 2 == 0:
                    # DVE square must read the SBUF copy (one-PSUM-input rule)
                    nc.vector.tensor_mul(sq[:, :cw], dst[:, m, c0:c0 + cw],
                                         dst[:, m, c0:c0 + cw])
                else:
                    nc.scalar.square(sq[:, :cw], ps[:, :cw])
                nc.tensor.matmul(
                    ssq[:, :cw], onesc_sb[:], sq[:, :cw],
                    start=(m == 0), stop=(m == ND - 1),
                )

            def proj_normed(w_dram, dst, chunks, tok0, scale, eps_sb,
                            first=False):
                """dst[:, m, c] = rstd * (x @ w)^T, rstd from raw sum-of-squares.

                rstd = Rsqrt(scale*ssq + eps): the k-side folds the uniform
                qk-norm-weight / sqrt(HD) score scale into (scale, eps)."""
                w_slices = []
                for kk in range(ND):
                    w_sl = wp.tile([128, DIM], F32R, tag="wslice")
                    nc.sync.dma_start(w_sl[:], w_dram[kk * 128:(kk + 1) * 128, :])
                    if first:
                        # interleave xt loads so the kk-outer first chunk can
                        # start as soon as the first (w, xt) slice pair lands
                        nc.sync.dma_start(xt_sb[:, kk, :], xt[:, kk, :])
                    w_slices.append(w_sl)
                for ci, (c0, cw) in enumerate(chunks):
                    ssq = sqp.tile([1, 512], F32)
                    if first and ci == 0:
                        # kk-outer in m-blocks of 4: PE consumes DMA'd slices
                        # incrementally instead of waiting for all 16
                        for mb in range(0, ND, 4):
                            blk = []
                            for m in range(mb, mb + 4):
                                ps = pp.tile([128, 512], F32, tag="ps")
                                blk.append(ps)
                            for kk in range(ND):
                                for mi, m in enumerate(range(mb, mb + 4)):
                                    nc.tensor.matmul(
                                        blk[mi][:, :cw],
                                        w_slices[kk][:, m * 128:(m + 1) * 128],
                                        xt_sb[:, kk, tok0 + c0: tok0 + c0 + cw],
                                        start=(kk == 0), stop=(kk == ND - 1),
                                    )
                            for mi, m in enumerate(range(mb, mb + 4)):
                                drain_ps(dst, blk[mi], m, c0, cw, ssq)
                    else:
                        for m in range(ND):
                            ps = pp.tile([128, 512], F32, tag="ps")
                            for kk in range(ND):
                                nc.tensor.matmul(
                                    ps[:, :cw],
                                    w_slices[kk][:, m * 128:(m + 1) * 128],
                                    xt_sb[:, kk, tok0 + c0: tok0 + c0 + cw],
                                    start=(kk == 0), stop=(kk == ND - 1),
                                )
                            drain_ps(dst, ps, m, c0, cw, ssq)
                    sqt = scrp.tile([1, 512], F32, tag="sqt")
                    nc.scalar.activation(sqt[:, :cw], ssq[:, :cw], AF.Sqrt,
                                         bias=eps_sb[:], scale=scale)
                    rstd = rcpp.tile([1, 512], F32R, tag="rstd")
                    with nc.allow_low_precision(reason="f32r rstd for matmul"):
                        nc.vector.reciprocal(rstd[:, :cw], sqt[:, :cw])
                    rsb = bcp.tile([128, 512], F32)
                    nc.tensor.matmul(rsb[:, :cw], onesr_sb[:],
                                     rstd[:, :cw], start=True, stop=True)
                    # stage the broadcast in SBUF: frees the psum slot and
                    # keeps the 8 muls off the one-PSUM-operand path
                    rsb_sb = scrp.tile([128, 512], F32, tag="rsbsb")
                    nc.scalar.copy(rsb_sb[:, :cw], rsb[:, :cw])
                    for m in range(ND):
                        nc.vector.tensor_mul(dst[:, m, c0:c0 + cw],
                                             dst[:, m, c0:c0 + cw],
                                             rsb_sb[:, :cw])

            proj_normed(wk, kt_sb, K_CHUNKS, 0, kscale, epsk_sb, first=True)
            proj_normed(wq, qt_sb, Q_CHUNKS, HALO, 1.0 / DIM, epsq_sb)

            # v: banded per-chunk tiles [key-in-window, dim]; unaligned
            # token windows of xT are free-axis slices (stationary operand)
            wv_slices = []
            for kk in range(ND):
                w_sl = wp.tile([128, DIM], F32R, tag="wslice")
                nc.sync.dma_start(w_sl[:], wv[kk * 128:(kk + 1) * 128, :])
                wv_slices.append(w_sl)
            for c in range(NCH):
                w0, ww, _, _ = _chunk_geom(c)
                for nn in range(2):
                    ps = pp.tile([128, 512], F32)
                    for kk in range(ND):
                        nc.tensor.matmul(
                            ps[:ww, :],
                            xt_sb[:, kk, w0:w0 + ww],
                            wv_slices[kk][:, nn * 512:(nn + 1) * 512],
                            start=(kk == 0), stop=(kk == ND - 1),
                        )
                    if (2 * c + nn) % 2 == 0:
                        nc.scalar.copy(v_sb[:ww, c, nn * 512:(nn + 1) * 512],
                                       ps[:ww, :])
                    else:
                        nc.vector.tensor_copy(v_sb[:ww, c, nn * 512:(nn + 1) * 512],
                                              ps[:ww, :])


def self_phase2(tc, nc, kt_sb, qt_sb, v_sb, ones128, relg_sb, relt_sb,
                wo, out):
        with (
            tc.tile_pool(name="wo", bufs=1) as wop,
            tc.tile_pool(name="atp", bufs=1) as atp,
            tc.tile_pool(name="exp", bufs=int(os.environ.get("KP_EXP", 4))) as expp,
            tc.tile_pool(name="rcs", bufs=int(os.environ.get("KP_RCS", 4))) as rcsp,
            tc.tile_pool(name="outp", bufs=3) as outp,
            tc.tile_pool(name="sps", bufs=int(os.environ.get("KP_SPS", 3)),
                         space="PSUM") as sps,
            tc.tile_pool(name="ytp", bufs=int(os.environ.get("KP_YTP", 2)),
                         space="PSUM") as ytp,
            tc.tile_pool(name="dbp", bufs=int(os.environ.get("KP_DBP", 1)),
                         space="PSUM") as dbp,
            tc.tile_pool(name="pso", bufs=int(os.environ.get("KP_PSO", 2)),
                         space="PSUM") as psop,
        ):
            wo_sb = wop.tile([128, ND, DIM], F32R)
            for hd in range(ND):
                nc.sync.dma_start(wo_sb[:, hd, :], wo[hd * 128:(hd + 1) * 128, :])
            aT = atp.tile([128, ND, QTOK], F32R)

            def stage_a(c):
                """QK scores + bias/mask + exp for chunk c; returns ex tiles."""
                w0, ww, q0, qw = _chunk_geom(c)
                ex_tiles = []
                for g in range(2):
                    st = sps.tile([ww, 4, qw], F32, tag="st")
                    for i in range(4):
                        h = 4 * g + i
                        nc.tensor.matmul(
                            st[:, i, :],
                            kt_sb[:, h, w0:w0 + ww],
                            qt_sb[:, h, q0:q0 + qw],
                            start=True, stop=True,
                        )
                    if c == 9:
                        rel = relt_sb[:, 4 * g:4 * g + 4, :]
                    else:
                        var = 0 if c == 0 else 1
                        rel = relg_sb[:, var, 4 * g:4 * g + 4, :]
                    # scores += slope_h * rel (slope prefolded; -1e9 at masked)
                    nc.vector.tensor_add(st[:], st[:], rel)
                    ex = expp.tile([ww, 4, qw], F32R, tag="ex")
                    nc.scalar.activation(ex[:], st[:], AF.Exp)
                    ex_tiles.append(ex)
                return ex_tiles

            def stage_b(c, ex_tiles):
                """PV + broadcast-denominator + normalize into aT for chunk c."""
                w0, ww, q0, qw = _chunk_geom(c)
                for g in range(2):
                    ex = ex_tiles[g]
                    yT = ytp.tile([128, 4, qw], F32, tag="yT")
                    for i in range(4):
                        h = 4 * g + i
                        nc.tensor.matmul(
                            yT[:, i, :],
                            v_sb[:ww, c, h * 128:(h + 1) * 128],
                            ex[:, i, :],
                            start=True, stop=True,
                        )
                    # denominator, already broadcast: ones^T[128] x ex
                    db = dbp.tile([128, 4, qw], F32, tag="db")
                    nc.tensor.matmul(db[:], ones128[:ww, :], ex[:, 0:4, :],
                                     start=True, stop=True)
                    rcp = rcsp.tile([128, 4, qw], F32, tag="rcp")
                    nc.vector.reciprocal(rcp[:], db[:])
                    nc.vector.tensor_mul(aT[:, 4 * g:4 * g + 4, q0:q0 + qw],
                                         yT[:], rcp[:])

            def wo_proj(t2):
                for nn in range(2):
                    ps_o = psop.tile([128, 512], F32)
                    for hd in range(ND):
                        nc.tensor.matmul(
                            ps_o[:],
                            aT[:, hd, t2 * 128:(t2 + 1) * 128],
                            wo_sb[:, hd, nn * 512:(nn + 1) * 512],
                            start=(hd == 0), stop=(hd == ND - 1),
                        )
                    o_sb = outp.tile([128, 512], F32, tag="osb")
                    nc.vector.tensor_copy(o_sb[:], ps_o[:])
                    nc.sync.dma_start(
                        out[t2 * 128:(t2 + 1) * 128, nn * 512:(nn + 1) * 512],
                        o_sb[:],
                    )

            # aT token tile t2 is complete after chunk ceil((128*t2+16)/112)
            done_t2 = {}
            for t2 in range(8):
                done_t2.setdefault(math.ceil((128 * t2 + 16) / QW), []).append(t2)

            # software pipeline: QK/exp of chunk c overlaps PV/norm of c-1
            ex_prev = stage_a(0)
            for c in range(1, NCH):
                ex_cur = stage_a(c)
                stage_b(c - 1, ex_prev)
                ex_prev = ex_cur
                for t2 in done_t2.get(c - 1, []):
                    wo_proj(t2)
            stage_b(NCH - 1, ex_prev)
            for t2 in done_t2.get(NCH - 1, []):
                wo_proj(t2)


def _host_constants():
    """relg [128, 2, H, QW]: bias tiles (slope folded, NEG at invalid).
    var 0 = chunk-0 (first-half cores mask the 16 zero-halo keys), var 1 =
    regular chunks. reltail [32, H, 16] for the 16-query tail chunk."""
    slopes = np.array([2.0 ** (-i) for i in range(H)], dtype=np.float64)

    kj = np.arange(128)[:, None, None]
    qi = np.arange(QW)[None, None, :]
    rel = kj - HALO - qi                      # key_global - query_global
    valid = (rel <= 0) & (rel >= -WINDOW)
    bias = slopes[None, :, None] * rel
    reg = np.where(valid, bias, NEG).astype(np.float32)       # [128, H, QW]

    relg = np.empty((128, 2, H, QW), dtype=np.float32)
    relg[:, 1] = reg
    relg[:, 0] = reg                          # per-core var 0 fixed below

    kjt = np.arange(32)[:, None, None]
    qit = np.arange(16)[None, None, :]
    relt = kjt - HALO - qit
    validt = (relt <= 0) & (relt >= -WINDOW)
    biast = slopes[None, :, None] * relt
    reltail = np.where(validt, biast, NEG).astype(np.float32)  # [32, H, 16]
    return relg, reltail


def _make_in_maps(x, wq, wk, wv, wo, q_norm_w, k_norm_w):
    x = np.ascontiguousarray(np.asarray(x, dtype=np.float32))
    wq = np.ascontiguousarray(np.asarray(wq, dtype=np.float32))
    wk = np.ascontiguousarray(np.asarray(wk, dtype=np.float32))
    wv = np.ascontiguousarray(np.asarray(wv, dtype=np.float32))
    wo = np.ascontiguousarray(np.asarray(wo, dtype=np.float32))
    q_norm_w = np.asarray(q_norm_w, dtype=np.float32)
    k_norm_w = np.asarray(k_norm_w, dtype=np.float32)

    relg_base, reltail = _host_constants()

    in_maps = []
    for c in range(8):
        b, hf = c // 2, c % 2
        base = hf * (T // 2)
        xsh = np.zeros((TSH, DIM), dtype=np.float32)
        lo = base - HALO
        if lo < 0:
            xsh[HALO:] = x[b, base: base + QTOK]
        else:
            xsh[:] = x[b, lo: base + QTOK]
        xt_c = np.ascontiguousarray(
            xsh.T.reshape(ND, 128, TSH).transpose(1, 0, 2))
        relg = relg_base.copy()
        if hf == 0:
            relg[:HALO, 0, :, :] = NEG       # zero-halo keys are invalid
        in_maps.append({
            "xt": xt_c, "wq": wq, "wk": wk, "wv": wv, "wo": wo,
            "relg": relg, "reltail": reltail,
            "ones_sq": np.ones((128, 128), dtype=np.float32),
            "ones_row": np.ones((1, 128), dtype=np.float32),
            "ones_col": np.ones((128, 1), dtype=np.float32),
        })

    return in_maps


def kernel(x, wq, wk, wv, wo, q_norm_w, k_norm_w):
    q_norm_w = np.asarray(q_norm_w, dtype=np.float32)
    k_norm_w = np.asarray(k_norm_w, dtype=np.float32)
    qkw = (q_norm_w * k_norm_w).astype(np.float64)
    cval = float(qkw[0]) / math.sqrt(HD)
    assert np.allclose(qkw, qkw[0], rtol=1e-5), (
        "kernel assumes uniform q_norm_w*k_norm_w (score scale folds into "
        "the k-side Rsqrt)")
    kscale = 1.0 / (DIM * cval * cval)
    keps = EPS / (cval * cval)

    key = ("nc", round(kscale, 9), round(keps, 12))
    if key not in _CACHE:
        _CACHE.clear()
        _CACHE[key] = _build_program(kscale, keps)
    nc = _CACHE[key]
    _CACHE["nc"] = nc
    in_maps = _make_in_maps(x, wq, wk, wv, wo, q_norm_w, k_norm_w)
    _CACHE["in_maps"] = in_maps
    import time as _time
    last_err = None
    for attempt in range(3):
        try:
            res = run_bass_kernel_spmd(nc, in_maps, core_ids=list(range(8)))
            break
        except Exception as e:  # transient NRT/device wedges recover on retry
            last_err = e
            _time.sleep(10 * (attempt + 1))
    else:
        raise last_err

    out = np.empty((B, T, DIM), dtype=np.float32)
    for c in range(8):
        b, hf = c // 2, c % 2
        out[b, hf * QTOK:(hf + 1) * QTOK, :] = res.results[c]["out"]
    return out


# revision 14
# speedup vs baseline: 1.2440x; 1.1323x over previous
"""Trainium2 Bass kernel for nn_CodecAttention (sliding-window ALiBi attention).

Reference computation (B=4, T=2048, DIM=1024, H=8, HD=128, WINDOW=16):
    xq = rms_norm(x @ wq) ; xk = rms_norm(x @ wk) ; xv = x @ wv
    scores = q k^T / sqrt(HD) + alibi_bias  (causal + 16-token sliding window)
    out = softmax(scores) @ v  -> reshape -> @ wo

Sharding: 8 cores = (batch b, sequence half). Each core processes 1024 query
tokens plus a 16-token key/value halo (zeros for the first half), fully
locally -- the attention window (16) never crosses the halo, so no
collectives are needed.

Banded attention: queries are processed in chunks of QW=112; each chunk's
entire window [q0-16, q0+111] fits in ONE unaligned 128-wide key tile
(SBUF free-axis slices of the transposed kT/qT are unaligned-friendly, and
v is projected directly into per-chunk banded [key, dim] tiles using
unaligned free-slices of x^T as the stationary operand). Per chunk+head:
one QK matmul, one PV matmul; the softmax denominator is produced ALREADY
BROADCAST across partitions by a single matmul with an all-ones [128,128]
stationary. ALiBi bias + causal/window mask is one tensor_add with a
host-precomputed per-4-head tile (slope folded in, -1e9 at invalid),
running on the otherwise-idle GPSIMD engine.

Everything that streams through the PE runs in bf16 (full rate at any
moving width -- f32r drops to 1/4 rate below 256) with f32 PSUM
accumulation; rel-err stays ~1e-3 vs the 2e-2 gate. RMS norm: sum-of-
squares via ones-matmul over the psum drain, ACT Sqrt (the uniform
q_norm*k_norm/sqrt(HD) score scale folds into the k-side scale/bias since
rms_norm is uniform-scale invariant) + DVE reciprocal, broadcast via a K=1
matmul, applied with bf16 2x-rate DVE muls. Weight DMAs issue from the
idle Pool sequencer (25ns/issue vs 565 on SP) in parallel with xt on SP.
"""

import math
import os

import numpy as np
from ml_dtypes import bfloat16 as np_bf16

os.environ.setdefault("MYCRO_LOCAL_CACHE", "1")

import concourse.mybir as mybir
import concourse.tile as tile
from concourse import bacc
from concourse.bass_utils import run_bass_kernel_spmd

F32 = mybir.dt.float32
F32R = mybir.dt.float32r
BF16 = mybir.dt.bfloat16
AF = mybir.ActivationFunctionType
ALU = mybir.AluOpType

B, T, DIM = 4, 2048, 1024
H, HD = 8, 128
WINDOW = 16
EPS = 1e-6
NEG = -1.0e9

HALO = 16                  # key/value halo tokens per shard
TSH = HALO + T // 2        # 1040 tokens per shard
QTOK = T // 2              # 1024 query tokens per shard
ND = DIM // 128            # 8 dim tiles
QW = 112                   # attention query-chunk width (window fits 128 keys)
NCH = 10                   # 9 full chunks + 1 tail chunk (16 queries)
K_CHUNKS = [(0, 384), (384, 384), (768, 272)]        # kT projection chunks
Q_CHUNKS = [(0, 512), (512, 512)]                    # qT projection chunks

POOL_ADD = os.environ.get("KP_POOL_ADD", "1") == "1"
POOL_WDMA = os.environ.get("KP_POOL_WDMA", "1") == "1"

_CACHE = {}


def _chunk_geom(c):
    """(k-window col start, k width, query col start in qt, query width)."""
    if c < 9:
        return 112 * c, 128, 112 * c, 112
    return 1008, 32, 1008, 16


def _build_program(kscale, keps):
    nc = bacc.Bacc("TRN2", debug=False, target_bir_lowering=False, num_devices=8)

    xt = nc.declare_dram_parameter("xt", [128, ND, TSH], BF16, isOutput=False)
    wq = nc.declare_dram_parameter("wq", [DIM, DIM], BF16, isOutput=False)
    wk = nc.declare_dram_parameter("wk", [DIM, DIM], BF16, isOutput=False)
    wv = nc.declare_dram_parameter("wv", [DIM, DIM], BF16, isOutput=False)
    wo = nc.declare_dram_parameter("wo", [DIM, DIM], F32R, isOutput=False)
    relg = nc.declare_dram_parameter("relg", [128, 2, H, QW], BF16, isOutput=False)
    reltail = nc.declare_dram_parameter("reltail", [32, H, 16], BF16, isOutput=False)
    ones_sq = nc.declare_dram_parameter("ones_sq", [128, 128], BF16, isOutput=False)
    ones_row = nc.declare_dram_parameter("ones_row", [1, 128], F32R, isOutput=False)
    ones_col = nc.declare_dram_parameter("ones_col", [128, 1], BF16, isOutput=False)
    out = nc.declare_dram_parameter("out", [QTOK, DIM], F32, isOutput=True)

    wdma = nc.gpsimd if POOL_WDMA else nc.sync

    with tile.TileContext(nc) as tc:
        with tc.tile_pool(name="big", bufs=1) as big:
            # ---- constants + persistent tensors (live for the whole kernel) ----
            kt_sb = big.tile([128, ND, TSH], BF16)
            qt_sb = big.tile([128, ND, QTOK], BF16)
            v_sb = big.tile([128, NCH, DIM], BF16)
            ones128 = big.tile([128, 128], BF16)
            onesr_sb = big.tile([1, 128], F32R)
            onesc_sb = big.tile([128, 1], BF16)
            relg_sb = big.tile([128, 2, H, QW], BF16)
            relt_sb = big.tile([32, H, 16], BF16)
            epsq_sb = big.tile([1, 1], F32)
            epsk_sb = big.tile([1, 1], F32)
            nc.vector.memset(epsq_sb[:], EPS)
            nc.vector.memset(epsk_sb[:], keps)
            # tiny, needed by the first ssq/bcast: head of the Pool DMA queue
            wdma.dma_start(onesc_sb[:], ones_col[:])
            wdma.dma_start(onesr_sb[:], ones_row[:])

            with nc.allow_low_precision(reason="bf16 pipeline, f32 psum"):
                self_phase1(tc, nc, wdma, kt_sb, qt_sb, v_sb, onesr_sb,
                            onesc_sb, epsq_sb, epsk_sb, kscale, xt, wq, wk, wv)
                # phase-2 constants: after all phase-1 DMAs in queue order,
                # still ~50us before first use
                nc.sync.dma_start(relg_sb[:], relg[:])
                nc.sync.dma_start(relt_sb[:], reltail[:])
                nc.sync.dma_start(ones128[:], ones_sq[:])
                self_phase2(tc, nc, kt_sb, qt_sb, v_sb, ones128,
                            relg_sb, relt_sb, wo, out)
    nc.compile()
    return nc


def self_phase1(tc, nc, wdma, kt_sb, qt_sb, v_sb, onesr_sb, onesc_sb,
                epsq_sb, epsk_sb, kscale, xt, wq, wk, wv):
    with (
        tc.tile_pool(name="xtp", bufs=1) as xtp,
        tc.tile_pool(name="wp", bufs=int(os.environ.get("KP_WP", 8))) as wp,
        tc.tile_pool(name="scr", bufs=2) as scrp,
        tc.tile_pool(name="rcp", bufs=2) as rcpp,
        tc.tile_pool(name="pp", bufs=int(os.environ.get("KP_PP", 6)),
                     space="PSUM") as pp,
        tc.tile_pool(name="sqp", bufs=int(os.environ.get("KP_SQP", 1)),
                     space="PSUM") as sqp,
        tc.tile_pool(name="bcp", bufs=int(os.environ.get("KP_BCP", 1)),
                     space="PSUM") as bcp,
    ):
            xt_sb = xtp.tile([128, ND, TSH], BF16)

            # ---- projections: kT and qT (with RMS-norm), v (banded tiles) ----
            def drain_ps(dst, ps, m, c0, cw, ssq):
                # ACT copies the raw psum out (bf16); DVE squares the bf16
                # copy at 2x rate; PE accumulates the token sum-of-squares
                nc.scalar.copy(dst[:, m, c0:c0 + cw], ps[:, :cw])
                sq = scrp.tile([128, 512], BF16, tag="sq")
                nc.vector.tensor_mul(sq[:, :cw], dst[:, m, c0:c0 + cw],
                                     dst[:, m, c0:c0 + cw])
                nc.tensor.matmul(
                    ssq[:, :cw], onesc_sb[:], sq[:, :cw],
                    start=(m == 0), stop=(m == ND - 1),
                )

            def proj_normed(w_dram, dst, chunks, tok0, scale, eps_sb,
                            first=False):
                """dst[:, m, c] = rstd * (x @ w)^T, rstd from raw sum-of-squares.

                rstd = 1/Sqrt(scale*ssq + eps): the k-side folds the uniform
                qk-norm-weight / sqrt(HD) score scale into (scale, eps)."""
                w_slices = []
                for kk in range(ND):
                    w_sl = wp.tile([128, DIM], BF16, tag="wslice")
                    wdma.dma_start(w_sl[:], w_dram[kk * 128:(kk + 1) * 128, :])
                    if first:
                        # interleave xt loads so the kk-outer first chunk can
                        # start as soon as the first (w, xt) slice pair lands
                        nc.sync.dma_start(xt_sb[:, kk, :], xt[:, kk, :])
                    w_slices.append(w_sl)
                for ci, (c0, cw) in enumerate(chunks):
                    ssq = sqp.tile([1, 512], F32)
                    if first and ci == 0:
                        # kk-outer in m-blocks of 4: PE consumes DMA'd slices
                        # incrementally instead of waiting for all 16
                        for mb in range(0, ND, 4):
                            blk = []
                            for m in range(mb, mb + 4):
                                ps = pp.tile([128, 512], F32, tag="ps")
                                blk.append(ps)
                            for kk in range(ND):
                                for mi, m in enumerate(range(mb, mb + 4)):
                                    nc.tensor.matmul(
                                        blk[mi][:, :cw],
                                        w_slices[kk][:, m * 128:(m + 1) * 128],
                                        xt_sb[:, kk, tok0 + c0: tok0 + c0 + cw],
                                        start=(kk == 0), stop=(kk == ND - 1),
                                    )
                            for mi, m in enumerate(range(mb, mb + 4)):
                                drain_ps(dst, blk[mi], m, c0, cw, ssq)
                    else:
                        for m in range(ND):
                            ps = pp.tile([128, 512], F32, tag="ps")
                            for kk in range(ND):
                                nc.tensor.matmul(
                                    ps[:, :cw],
                                    w_slices[kk][:, m * 128:(m + 1) * 128],
                                    xt_sb[:, kk, tok0 + c0: tok0 + c0 + cw],
                                    start=(kk == 0), stop=(kk == ND - 1),
                                )
                            drain_ps(dst, ps, m, c0, cw, ssq)
                    sqt = scrp.tile([1, 512], F32, tag="sqt")
                    nc.scalar.activation(sqt[:, :cw], ssq[:, :cw], AF.Sqrt,
                                         bias=eps_sb[:], scale=scale)
                    rstd = rcpp.tile([1, 512], F32R, tag="rstd")
                    nc.vector.reciprocal(rstd[:, :cw], sqt[:, :cw])
                    rsb = bcp.tile([128, 512], F32)
                    nc.tensor.matmul(rsb[:, :cw], onesr_sb[:],
                                     rstd[:, :cw], start=True, stop=True)
                    # stage the broadcast in SBUF bf16: the 8 muls then run
                    # all-2-byte at DVE 2x rate
                    rsb_sb = scrp.tile([128, 512], BF16, tag="rsbsb")
                    nc.scalar.copy(rsb_sb[:, :cw], rsb[:, :cw])
                    npool = int(os.environ.get("KP_POOL_MUL", 3))
                    for m in range(ND):
                        # SBUF-only bf16 muls: split between DVE (2x mode)
                        # and the otherwise-idle GPSIMD engine
                        eng = nc.gpsimd if m >= ND - npool else nc.vector
                        eng.tensor_mul(dst[:, m, c0:c0 + cw],
                                       dst[:, m, c0:c0 + cw],
                                       rsb_sb[:, :cw])

            proj_normed(wk, kt_sb, K_CHUNKS, 0, kscale, epsk_sb, first=True)
            proj_normed(wq, qt_sb, Q_CHUNKS, HALO, 1.0 / DIM, epsq_sb)

            # v: banded per-chunk tiles [key-in-window, dim]; unaligned
            # token windows of xT are free-axis slices (stationary operand)
            wv_slices = []
            for kk in range(ND):
                w_sl = wp.tile([128, DIM], BF16, tag="wslice")
                wdma.dma_start(w_sl[:], wv[kk * 128:(kk + 1) * 128, :])
                wv_slices.append(w_sl)
            for c in range(NCH):
                w0, ww, _, _ = _chunk_geom(c)
                for nn in range(2):
                    ps = pp.tile([128, 512], F32)
                    for kk in range(ND):
                        nc.tensor.matmul(
                            ps[:ww, :],
                            xt_sb[:, kk, w0:w0 + ww],
                            wv_slices[kk][:, nn * 512:(nn + 1) * 512],
                            start=(kk == 0), stop=(kk == ND - 1),
                        )
                    if (2 * c + nn) % 2 == 0:
                        nc.scalar.copy(v_sb[:ww, c, nn * 512:(nn + 1) * 512],
                                       ps[:ww, :])
                    else:
                        nc.vector.tensor_copy(v_sb[:ww, c, nn * 512:(nn + 1) * 512],
                                              ps[:ww, :])


def self_phase2(tc, nc, kt_sb, qt_sb, v_sb, ones128, relg_sb, relt_sb,
                wo, out):
        with (
            tc.tile_pool(name="wo", bufs=1) as wop,
            tc.tile_pool(name="atp", bufs=1) as atp,
            tc.tile_pool(name="exp", bufs=int(os.environ.get("KP_EXP", 4))) as expp,
            tc.tile_pool(name="rcs", bufs=int(os.environ.get("KP_RCS", 4))) as rcsp,
            tc.tile_pool(name="outp", bufs=3) as outp,
            tc.tile_pool(name="sps", bufs=int(os.environ.get("KP_SPS", 3)),
                         space="PSUM") as sps,
            tc.tile_pool(name="ytp", bufs=int(os.environ.get("KP_YTP", 2)),
                         space="PSUM") as ytp,
            tc.tile_pool(name="dbp", bufs=int(os.environ.get("KP_DBP", 1)),
                         space="PSUM") as dbp,
            tc.tile_pool(name="pso", bufs=int(os.environ.get("KP_PSO", 2)),
                         space="PSUM") as psop,
        ):
            wo_sb = wop.tile([128, ND, DIM], F32R)
            for hd in range(ND):
                nc.sync.dma_start(wo_sb[:, hd, :], wo[hd * 128:(hd + 1) * 128, :])
            aT = atp.tile([128, ND, QTOK], F32R)

            def stage_a(c):
                """QK scores + bias/mask + exp for chunk c; returns ex tiles."""
                w0, ww, q0, qw = _chunk_geom(c)
                ex_tiles = []
                for g in range(2):
                    st = sps.tile([ww, 4, qw], F32, tag="st")
                    for i in range(4):
                        h = 4 * g + i
                        nc.tensor.matmul(
                            st[:, i, :],
                            kt_sb[:, h, w0:w0 + ww],
                            qt_sb[:, h, q0:q0 + qw],
                            start=True, stop=True,
                        )
                    if c == 9:
                        wmask = relt_sb[:, 4 * g:4 * g + 4, :]
                    else:
                        var = 0 if c == 0 else 1
                        wmask = relg_sb[:, var, 4 * g:4 * g + 4, :]
                    # exp(s + bias) = exp(s) * exp(bias): the host-built
                    # wmask tile (0 at masked) turns the bias-add into an
                    # all-bf16 SBUF mul that runs at DVE 2x rate
                    ex = expp.tile([ww, 4, qw], BF16, tag="ex")
                    nc.scalar.activation(ex[:], st[:], AF.Exp)
                    nc.vector.tensor_mul(ex[:], ex[:], wmask)
                    ex_tiles.append(ex)
                return ex_tiles

            def stage_b(c, ex_tiles):
                """PV + broadcast-denominator + normalize into aT for chunk c."""
                w0, ww, q0, qw = _chunk_geom(c)
                for g in range(2):
                    ex = ex_tiles[g]
                    yT = ytp.tile([128, 4, qw], F32, tag="yT")
                    for i in range(4):
                        h = 4 * g + i
                        nc.tensor.matmul(
                            yT[:, i, :],
                            v_sb[:ww, c, h * 128:(h + 1) * 128],
                            ex[:, i, :],
                            start=True, stop=True,
                        )
                    # denominator, already broadcast: ones^T[128] x ex
                    db = dbp.tile([128, 4, qw], F32, tag="db")
                    nc.tensor.matmul(db[:], ones128[:ww, :], ex[:, 0:4, :],
                                     start=True, stop=True)
                    rcp = rcsp.tile([128, 4, qw], F32, tag="rcp")
                    nc.vector.reciprocal(rcp[:], db[:])
                    nc.vector.tensor_mul(aT[:, 4 * g:4 * g + 4, q0:q0 + qw],
                                         yT[:], rcp[:])

            def wo_proj(t2):
                for nn in range(2):
                    ps_o = psop.tile([128, 512], F32)
                    for hd in range(ND):
                        nc.tensor.matmul(
                            ps_o[:],
                            aT[:, hd, t2 * 128:(t2 + 1) * 128],
                            wo_sb[:, hd, nn * 512:(nn + 1) * 512],
                            start=(hd == 0), stop=(hd == ND - 1),
                        )
                    o_sb = outp.tile([128, 512], F32, tag="osb")
                    nc.scalar.copy(o_sb[:], ps_o[:])
                    nc.sync.dma_start(
                        out[t2 * 128:(t2 + 1) * 128, nn * 512:(nn + 1) * 512],
                        o_sb[:],
                    )

            # aT token tile t2 is complete after chunk ceil((128*t2+16)/112)
            done_t2 = {}
            for t2 in range(8):
                done_t2.setdefault(math.ceil((128 * t2 + 16) / QW), []).append(t2)

            # software pipeline: QK/exp of chunk c overlaps PV/norm of c-1
            ex_prev = stage_a(0)
            for c in range(1, NCH):
                ex_cur = stage_a(c)
                stage_b(c - 1, ex_prev)
                ex_prev = ex_cur
                for t2 in done_t2.get(c - 1, []):
                    wo_proj(t2)
            stage_b(NCH - 1, ex_prev)
            for t2 in done_t2.get(NCH - 1, []):
                wo_proj(t2)


def _host_constants():
    """relg [128, 2, H, QW]: multiplicative masks exp(slope_h*rel), 0 at
    invalid positions (exp(s+bias) = exp(s)*exp(bias)). var 0 = chunk-0
    (first-half cores mask the 16 zero-halo keys), var 1 = regular chunks.
    reltail [32, H, 16] for the 16-query tail chunk."""
    slopes = np.array([2.0 ** (-i) for i in range(H)], dtype=np.float64)

    kj = np.arange(128)[:, None, None]
    qi = np.arange(QW)[None, None, :]
    rel = kj - HALO - qi                      # key_global - query_global
    valid = (rel <= 0) & (rel >= -WINDOW)
    bias = slopes[None, :, None] * rel
    reg = np.where(valid, np.exp(bias), 0.0).astype(np_bf16)  # [128, H, QW]

    relg = np.empty((128, 2, H, QW), dtype=np_bf16)
    relg[:, 1] = reg
    relg[:, 0] = reg                          # per-core var 0 fixed below

    kjt = np.arange(32)[:, None, None]
    qit = np.arange(16)[None, None, :]
    relt = kjt - HALO - qit
    validt = (relt <= 0) & (relt >= -WINDOW)
    biast = slopes[None, :, None] * relt
    reltail = np.where(validt, np.exp(biast), 0.0).astype(np_bf16)
    return relg, reltail


def _make_in_maps(x, wq, wk, wv, wo, q_norm_w, k_norm_w):
    x = np.ascontiguousarray(np.asarray(x, dtype=np.float32))
    wq = np.asarray(wq, dtype=np.float32).astype(np_bf16)
    wk = np.asarray(wk, dtype=np.float32).astype(np_bf16)
    wv = np.asarray(wv, dtype=np.float32).astype(np_bf16)
    wo = np.ascontiguousarray(np.asarray(wo, dtype=np.float32))

    relg_base, reltail = _host_constants()

    in_maps = []
    for c in range(8):
        b, hf = c // 2, c % 2
        base = hf * (T // 2)
        xsh = np.zeros((TSH, DIM), dtype=np.float32)
        lo = base - HALO
        if lo < 0:
            xsh[HALO:] = x[b, base: base + QTOK]
        else:
            xsh[:] = x[b, lo: base + QTOK]
        xt_c = np.ascontiguousarray(
            xsh.T.reshape(ND, 128, TSH).transpose(1, 0, 2)).astype(np_bf16)
        relg = relg_base.copy()
        if hf == 0:
            relg[:HALO, 0, :, :] = 0         # zero-halo keys are invalid
        in_maps.append({
            "xt": xt_c, "wq": wq, "wk": wk, "wv": wv, "wo": wo,
            "relg": relg, "reltail": reltail,
            "ones_sq": np.ones((128, 128), dtype=np_bf16),
            "ones_row": np.ones((1, 128), dtype=np.float32),
            "ones_col": np.ones((128, 1), dtype=np_bf16),
        })

    return in_maps


def kernel(x, wq, wk, wv, wo, q_norm_w, k_norm_w):
    q_norm_w = np.asarray(q_norm_w, dtype=np.float32)
    k_norm_w = np.asarray(k_norm_w, dtype=np.float32)
    qkw = (q_norm_w * k_norm_w).astype(np.float64)
    cval = float(qkw[0]) / math.sqrt(HD)
    assert np.allclose(qkw, qkw[0], rtol=1e-5), (
        "kernel assumes uniform q_norm_w*k_norm_w (score scale folds into "
        "the k-side Sqrt scale/bias)")
    kscale = 1.0 / (DIM * cval * cval)
    keps = EPS / (cval * cval)

    key = ("nc", round(kscale, 9), round(keps, 12))
    if key not in _CACHE:
        _CACHE.clear()
        _CACHE[key] = _build_program(kscale, keps)
    nc = _CACHE[key]
    _CACHE["nc"] = nc
    in_maps = _make_in_maps(x, wq, wk, wv, wo, q_norm_w, k_norm_w)
    _CACHE["in_maps"] = in_maps
    import time as _time
    last_err = None
    for attempt in range(3):
        try:
            res = run_bass_kernel_spmd(nc, in_maps, core_ids=list(range(8)))
            break
        except Exception as e:  # transient NRT/device wedges recover on retry
            last_err = e
            _time.sleep(10 * (attempt + 1))
    else:
        raise last_err

    out = np.empty((B, T, DIM), dtype=np.float32)
    for c in range(8):
        b, hf = c // 2, c % 2
        out[b, hf * QTOK:(hf + 1) * QTOK, :] = res.results[c]["out"]
    return out


# revision 26
# speedup vs baseline: 1.3666x; 1.0986x over previous
"""Trainium2 Bass kernel for nn_CodecAttention (sliding-window ALiBi attention).

Reference computation (B=4, T=2048, DIM=1024, H=8, HD=128, WINDOW=16):
    xq = rms_norm(x @ wq) ; xk = rms_norm(x @ wk) ; xv = x @ wv
    scores = q k^T / sqrt(HD) + alibi_bias  (causal + 16-token sliding window)
    out = softmax(scores) @ v  -> reshape -> @ wo

Sharding: 8 cores = (batch b, sequence half). Each core processes 1024 query
tokens plus a 16-token key/value halo (zeros for the first half), fully
locally -- the attention window (16) never crosses the halo, so no
collectives are needed.

Banded attention: queries are processed in chunks of QW=112; each chunk's
entire window [q0-16, q0+111] fits in ONE unaligned 128-wide key tile
(SBUF free-axis slices of the transposed kT/qT are unaligned-friendly, and
v is projected directly into per-chunk banded [key, dim] tiles using
unaligned free-slices of x^T as the stationary operand). Per chunk+head:
one QK matmul, one PV matmul; the softmax denominator is produced ALREADY
BROADCAST across partitions by a single matmul with an all-ones [128,128]
stationary. ALiBi bias + causal/window mask is one tensor_add with a
host-precomputed per-4-head tile (slope folded in, -1e9 at invalid),
running on the otherwise-idle GPSIMD engine.

Everything that streams through the PE runs in bf16 (full rate at any
moving width -- f32r drops to 1/4 rate below 256) with f32 PSUM
accumulation; rel-err stays ~1e-3 vs the 2e-2 gate. RMS norm: sum-of-
squares via ones-matmul over the psum drain, ACT Sqrt (the uniform
q_norm*k_norm/sqrt(HD) score scale folds into the k-side scale/bias since
rms_norm is uniform-scale invariant) + DVE reciprocal, broadcast via a K=1
matmul, applied with bf16 2x-rate DVE muls. Weight DMAs issue from the
idle Pool sequencer (25ns/issue vs 565 on SP) in parallel with xt on SP.
"""

import math
import os

import numpy as np
from ml_dtypes import bfloat16 as np_bf16

os.environ.setdefault("MYCRO_LOCAL_CACHE", "1")

import concourse.mybir as mybir
import concourse.tile as tile
from concourse import bacc
from concourse.bass_utils import run_bass_kernel_spmd

F32 = mybir.dt.float32
F32R = mybir.dt.float32r
BF16 = mybir.dt.bfloat16
AF = mybir.ActivationFunctionType
ALU = mybir.AluOpType

B, T, DIM = 4, 2048, 1024
H, HD = 8, 128
WINDOW = 16
EPS = 1e-6
NEG = -1.0e9

HALO = 16                  # key/value halo tokens per shard
TSH = HALO + T // 2        # 1040 tokens per shard
QTOK = T // 2              # 1024 query tokens per shard
ND = DIM // 128            # 8 dim tiles
QW = 112                   # attention query-chunk width (window fits 128 keys)
NCH = 10                   # 9 full chunks + 1 tail chunk (16 queries)
K_CHUNKS = [(0, 384), (384, 384), (768, 272)]        # kT projection chunks
Q_CHUNKS = [(0, 512), (512, 512)]                    # qT projection chunks

POOL_ADD = os.environ.get("KP_POOL_ADD", "1") == "1"
POOL_WDMA = os.environ.get("KP_POOL_WDMA", "1") == "1"

_CACHE = {}


def _chunk_geom(c):
    """(k-window col start, k width, query col start in qt, query width)."""
    if c < 9:
        return 112 * c, 128, 112 * c, 112
    return 1008, 32, 1008, 16


def _build_program(kscale, keps):
    nc = bacc.Bacc("TRN2", debug=False, target_bir_lowering=False, num_devices=8)

    xt = nc.declare_dram_parameter("xt", [128, ND, TSH], BF16, isOutput=False)
    wq = nc.declare_dram_parameter("wq", [DIM, DIM], BF16, isOutput=False)
    wk = nc.declare_dram_parameter("wk", [DIM, DIM], BF16, isOutput=False)
    wv = nc.declare_dram_parameter("wv", [DIM, DIM], BF16, isOutput=False)
    wo = nc.declare_dram_parameter("wo", [DIM, DIM], BF16, isOutput=False)
    relg = nc.declare_dram_parameter("relg", [128, 2, H, QW], BF16, isOutput=False)
    reltail = nc.declare_dram_parameter("reltail", [32, H, 16], BF16, isOutput=False)
    ones_sq = nc.declare_dram_parameter("ones_sq", [128, 128], BF16, isOutput=False)
    ones_row = nc.declare_dram_parameter("ones_row", [1, 128], F32R, isOutput=False)
    ones_col = nc.declare_dram_parameter("ones_col", [128, 1], F32R, isOutput=False)
    out = nc.declare_dram_parameter("out", [QTOK, DIM], F32, isOutput=True)

    wdma = nc.gpsimd if POOL_WDMA else nc.sync

    with tile.TileContext(nc) as tc:
        with tc.tile_pool(name="big", bufs=1) as big:
            # ---- constants + persistent tensors (live for the whole kernel) ----
            kt_sb = big.tile([128, ND, TSH], BF16)
            qt_sb = big.tile([128, ND, QTOK], BF16)
            v_sb = big.tile([128, NCH, DIM], BF16)
            ones128 = big.tile([128, 128], BF16)
            onesr_sb = big.tile([1, 128], F32R)
            onesc_sb = big.tile([128, 1], F32R)
            relg_sb = big.tile([128, 2, H, QW], BF16)
            relt_sb = big.tile([32, H, 16], BF16)
            epsq_sb = big.tile([1, 1], F32)
            epsk_sb = big.tile([1, 1], F32)
            nc.vector.memset(epsq_sb[:], EPS)
            nc.vector.memset(epsk_sb[:], keps)
            # tiny, needed by the first ssq/bcast: head of the Pool DMA queue
            wdma.dma_start(onesc_sb[:], ones_col[:])
            wdma.dma_start(onesr_sb[:], ones_row[:])

            wo_sb = big.tile([128, ND, DIM], BF16)

            with nc.allow_low_precision(reason="bf16 pipeline, f32 psum"):
                self_phase1(tc, nc, wdma, kt_sb, qt_sb, v_sb, wo_sb, onesr_sb,
                            onesc_sb, epsq_sb, epsk_sb, kscale, xt, wq, wk, wv,
                            wo, relg_sb, relg, relt_sb, reltail, ones128,
                            ones_sq)
                self_phase2(tc, nc, kt_sb, qt_sb, v_sb, wo_sb, ones128,
                            relg_sb, relt_sb, out)
    nc.compile()
    return nc


def self_phase1(tc, nc, wdma, kt_sb, qt_sb, v_sb, wo_sb, onesr_sb, onesc_sb,
                epsq_sb, epsk_sb, kscale, xt, wq, wk, wv, wo,
                relg_sb, relg, relt_sb, reltail, ones128, ones_sq):
    with (
        tc.tile_pool(name="xtp", bufs=1) as xtp,
        tc.tile_pool(name="wp", bufs=int(os.environ.get("KP_WP", 24))) as wp,
        tc.tile_pool(name="scr", bufs=int(os.environ.get("KP_SCR", 4))) as scrp,
        tc.tile_pool(name="rcp", bufs=2) as rcpp,
        tc.tile_pool(name="pp", bufs=int(os.environ.get("KP_PP", 6)),
                     space="PSUM") as pp,
        tc.tile_pool(name="sqp", bufs=int(os.environ.get("KP_SQP", 1)),
                     space="PSUM") as sqp,
        tc.tile_pool(name="bcp", bufs=int(os.environ.get("KP_BCP", 1)),
                     space="PSUM") as bcp,
    ):
            xt_sb = xtp.tile([128, ND, TSH], BF16)

            # ---- prefetch: ALL weights up front on the Pool queue (in
            # parallel with xt on SP) so no DMA ever gates mid-kernel
            # compute; bf16 slices are small enough to all live in SBUF.
            # Pool compute (rstd muls) only gates attention, so it can
            # queue behind the descriptor generation.
            def w_load(w_dram, interleave_xt=False):
                slices = []
                for kk in range(ND):
                    w_sl = wp.tile([128, DIM], BF16, tag="wslice")
                    wdma.dma_start(w_sl[:], w_dram[kk * 128:(kk + 1) * 128, :])
                    if interleave_xt:
                        # xt interleaves so the kk-outer first chunk starts
                        # as soon as the first (w, xt) slice pair lands
                        nc.sync.dma_start(xt_sb[:, kk, :], xt[:, kk, :])
                    slices.append(w_sl)
                return slices

            wk_sl = w_load(wk, interleave_xt=True)
            # phase-2 constants ride the SP queue right behind xt
            nc.sync.dma_start(relg_sb[:], relg[:])
            nc.sync.dma_start(relt_sb[:], reltail[:])
            nc.sync.dma_start(ones128[:], ones_sq[:])
            wq_sl = w_load(wq)
            wv_sl = w_load(wv)
            for hd in range(ND):
                wdma.dma_start(wo_sb[:, hd, :], wo[hd * 128:(hd + 1) * 128, :])

            # ---- projections: kT and qT (with RMS-norm), v (banded tiles) ----
            def drain_ps(dst, ps, m, c0, cw, acc):
                # ACT copies the raw psum out (bf16) and squares the psum;
                # DVE accumulates squares so ONE ones-matmul reduces ssq
                nc.scalar.copy(dst[:, m, c0:c0 + cw], ps[:, :cw])
                if m == 0:
                    nc.scalar.square(acc[:, :cw], ps[:, :cw])
                else:
                    sq = scrp.tile([128, 512], F32R, tag="sq")
                    nc.scalar.square(sq[:, :cw], ps[:, :cw])
                    nc.vector.tensor_add(acc[:, :cw], acc[:, :cw], sq[:, :cw])

            def proj_normed(w_slices, dst, chunks, tok0, scale, eps_sb,
                            first=False):
                """dst[:, m, c] = rstd * (x @ w)^T, rstd from raw sum-of-squares.

                rstd = 1/Sqrt(scale*ssq + eps): the k-side folds the uniform
                qk-norm-weight / sqrt(HD) score scale into (scale, eps)."""
                for ci, (c0, cw) in enumerate(chunks):
                    acc = scrp.tile([128, 512], F32R, tag="acc")
                    if first and ci == 0:
                        # kk-outer in m-blocks of 4: PE consumes DMA'd slices
                        # incrementally instead of waiting for all 16
                        for mb in range(0, ND, 4):
                            blk = []
                            for m in range(mb, mb + 4):
                                ps = pp.tile([128, 512], F32, tag="ps")
                                blk.append(ps)
                            for kk in range(ND):
                                for mi, m in enumerate(range(mb, mb + 4)):
                                    nc.tensor.matmul(
                                        blk[mi][:, :cw],
                                        w_slices[kk][:, m * 128:(m + 1) * 128],
                                        xt_sb[:, kk, tok0 + c0: tok0 + c0 + cw],
                                        start=(kk == 0), stop=(kk == ND - 1),
                                    )
                            for mi, m in enumerate(range(mb, mb + 4)):
                                drain_ps(dst, blk[mi], m, c0, cw, acc)
                    else:
                        for m in range(ND):
                            ps = pp.tile([128, 512], F32, tag="ps")
                            for kk in range(ND):
                                nc.tensor.matmul(
                                    ps[:, :cw],
                                    w_slices[kk][:, m * 128:(m + 1) * 128],
                                    xt_sb[:, kk, tok0 + c0: tok0 + c0 + cw],
                                    start=(kk == 0), stop=(kk == ND - 1),
                                )
                            drain_ps(dst, ps, m, c0, cw, acc)
                    ssq = sqp.tile([1, 512], F32)
                    nc.tensor.matmul(ssq[:, :cw], onesc_sb[:], acc[:, :cw],
                                     start=True, stop=True)
                    sqt = scrp.tile([1, 512], F32, tag="sqt")
                    nc.scalar.activation(sqt[:, :cw], ssq[:, :cw], AF.Sqrt,
                                         bias=eps_sb[:], scale=scale)
                    rstd = rcpp.tile([1, 512], F32R, tag="rstd")
                    nc.vector.reciprocal(rstd[:, :cw], sqt[:, :cw])
                    rsb = bcp.tile([128, 512], F32)
                    nc.tensor.matmul(rsb[:, :cw], onesr_sb[:],
                                     rstd[:, :cw], start=True, stop=True)
                    # stage the broadcast in SBUF bf16: the 8 muls then run
                    # all-2-byte at DVE 2x rate
                    rsb_sb = scrp.tile([128, 512], BF16, tag="rsbsb")
                    nc.scalar.copy(rsb_sb[:, :cw], rsb[:, :cw])
                    npool = int(os.environ.get("KP_POOL_MUL", 3))
                    for m in range(ND):
                        # SBUF-only bf16 muls: split between DVE (2x mode)
                        # and the otherwise-idle GPSIMD engine
                        eng = nc.gpsimd if m >= ND - npool else nc.vector
                        eng.tensor_mul(dst[:, m, c0:c0 + cw],
                                       dst[:, m, c0:c0 + cw],
                                       rsb_sb[:, :cw])

            proj_normed(wk_sl, kt_sb, K_CHUNKS, 0, kscale, epsk_sb, first=True)
            proj_normed(wq_sl, qt_sb, Q_CHUNKS, HALO, 1.0 / DIM, epsq_sb)

            # v: banded per-chunk tiles [key-in-window, dim]; unaligned
            # token windows of xT are free-axis slices (stationary operand)
            for c in range(NCH):
                w0, ww, _, _ = _chunk_geom(c)
                for nn in range(2):
                    ps = pp.tile([128, 512], F32)
                    for kk in range(ND):
                        nc.tensor.matmul(
                            ps[:ww, :],
                            xt_sb[:, kk, w0:w0 + ww],
                            wv_sl[kk][:, nn * 512:(nn + 1) * 512],
                            start=(kk == 0), stop=(kk == ND - 1),
                        )
                    if (2 * c + nn) % 2 == 0:
                        nc.scalar.copy(v_sb[:ww, c, nn * 512:(nn + 1) * 512],
                                       ps[:ww, :])
                    else:
                        nc.vector.tensor_copy(v_sb[:ww, c, nn * 512:(nn + 1) * 512],
                                              ps[:ww, :])


def self_phase2(tc, nc, kt_sb, qt_sb, v_sb, wo_sb, ones128, relg_sb,
                relt_sb, out):
        with (
            tc.tile_pool(name="atp", bufs=1) as atp,
            tc.tile_pool(name="exp", bufs=int(os.environ.get("KP_EXP", 4))) as expp,
            tc.tile_pool(name="rcs", bufs=int(os.environ.get("KP_RCS", 4))) as rcsp,
            tc.tile_pool(name="outp", bufs=3) as outp,
            tc.tile_pool(name="sps", bufs=int(os.environ.get("KP_SPS", 3)),
                         space="PSUM") as sps,
            tc.tile_pool(name="ytp", bufs=int(os.environ.get("KP_YTP", 2)),
                         space="PSUM") as ytp,
            tc.tile_pool(name="dbp", bufs=int(os.environ.get("KP_DBP", 1)),
                         space="PSUM") as dbp,
            tc.tile_pool(name="pso", bufs=int(os.environ.get("KP_PSO", 2)),
                         space="PSUM") as psop,
        ):
            aT = atp.tile([128, ND, QTOK], BF16)

            def stage_a(c):
                """QK scores + bias/mask + exp for chunk c; returns ex tiles."""
                w0, ww, q0, qw = _chunk_geom(c)
                ex_tiles = []
                for g in range(2):
                    st = sps.tile([ww, 4, qw], F32, tag="st")
                    for i in range(4):
                        h = 4 * g + i
                        nc.tensor.matmul(
                            st[:, i, :],
                            kt_sb[:, h, w0:w0 + ww],
                            qt_sb[:, h, q0:q0 + qw],
                            start=True, stop=True,
                        )
                    if c == 9:
                        wmask = relt_sb[:, 4 * g:4 * g + 4, :]
                    else:
                        var = 0 if c == 0 else 1
                        wmask = relg_sb[:, var, 4 * g:4 * g + 4, :]
                    # exp(s + bias) = exp(s) * exp(bias): the host-built
                    # wmask tile (0 at masked) turns the bias-add into an
                    # all-bf16 SBUF mul that runs at DVE 2x rate
                    ex = expp.tile([ww, 4, qw], BF16, tag="ex")
                    nc.scalar.activation(ex[:], st[:], AF.Exp)
                    nc.vector.tensor_mul(ex[:], ex[:], wmask)
                    ex_tiles.append(ex)
                return ex_tiles

            def stage_b(c, ex_tiles):
                """PV + broadcast-denominator + normalize into aT for chunk c."""
                w0, ww, q0, qw = _chunk_geom(c)
                for g in range(2):
                    ex = ex_tiles[g]
                    yT = ytp.tile([128, 4, qw], F32, tag="yT")
                    for i in range(4):
                        h = 4 * g + i
                        nc.tensor.matmul(
                            yT[:, i, :],
                            v_sb[:ww, c, h * 128:(h + 1) * 128],
                            ex[:, i, :],
                            start=True, stop=True,
                        )
                    # denominator, already broadcast: ones^T[128] x ex
                    db = dbp.tile([128, 4, qw], F32, tag="db")
                    nc.tensor.matmul(db[:], ones128[:ww, :], ex[:, 0:4, :],
                                     start=True, stop=True)
                    rcp = rcsp.tile([128, 4, qw], F32, tag="rcp")
                    nc.vector.reciprocal(rcp[:], db[:])
                    nc.vector.tensor_mul(aT[:, 4 * g:4 * g + 4, q0:q0 + qw],
                                         yT[:], rcp[:])

            def wo_proj(t2):
                for nn in range(2):
                    ps_o = psop.tile([128, 512], F32)
                    for hd in range(ND):
                        nc.tensor.matmul(
                            ps_o[:],
                            aT[:, hd, t2 * 128:(t2 + 1) * 128],
                            wo_sb[:, hd, nn * 512:(nn + 1) * 512],
                            start=(hd == 0), stop=(hd == ND - 1),
                        )
                    o_sb = outp.tile([128, 512], F32, tag="osb")
                    nc.scalar.copy(o_sb[:], ps_o[:])
                    nc.sync.dma_start(
                        out[t2 * 128:(t2 + 1) * 128, nn * 512:(nn + 1) * 512],
                        o_sb[:],
                    )

            # aT token tile t2 is complete after chunk ceil((128*t2+16)/112)
            done_t2 = {}
            for t2 in range(8):
                done_t2.setdefault(math.ceil((128 * t2 + 16) / QW), []).append(t2)

            # software pipeline: QK/exp of chunk c overlaps PV/norm of c-1
            ex_prev = stage_a(0)
            for c in range(1, NCH):
                ex_cur = stage_a(c)
                stage_b(c - 1, ex_prev)
                ex_prev = ex_cur
                for t2 in done_t2.get(c - 1, []):
                    wo_proj(t2)
            stage_b(NCH - 1, ex_prev)
            for t2 in done_t2.get(NCH - 1, []):
                wo_proj(t2)


def _host_constants():
    """relg [128, 2, H, QW]: multiplicative masks exp(slope_h*rel), 0 at
    invalid positions (exp(s+bias) = exp(s)*exp(bias)). var 0 = chunk-0
    (first-half cores mask the 16 zero-halo keys), var 1 = regular chunks.
    reltail [32, H, 16] for the 16-query tail chunk."""
    slopes = np.array([2.0 ** (-i) for i in range(H)], dtype=np.float64)

    kj = np.arange(128)[:, None, None]
    qi = np.arange(QW)[None, None, :]
    rel = kj - HALO - qi                      # key_global - query_global
    valid = (rel <= 0) & (rel >= -WINDOW)
    bias = slopes[None, :, None] * rel
    reg = np.where(valid, np.exp(bias), 0.0).astype(np_bf16)  # [128, H, QW]

    relg = np.empty((128, 2, H, QW), dtype=np_bf16)
    relg[:, 1] = reg
    relg[:, 0] = reg                          # per-core var 0 fixed below

    kjt = np.arange(32)[:, None, None]
    qit = np.arange(16)[None, None, :]
    relt = kjt - HALO - qit
    validt = (relt <= 0) & (relt >= -WINDOW)
    biast = slopes[None, :, None] * relt
    reltail = np.where(validt, np.exp(biast), 0.0).astype(np_bf16)
    return relg, reltail


def _make_in_maps(x, wq, wk, wv, wo, q_norm_w, k_norm_w):
    x = np.ascontiguousarray(np.asarray(x, dtype=np.float32))
    wq = np.asarray(wq, dtype=np.float32).astype(np_bf16)
    wk = np.asarray(wk, dtype=np.float32).astype(np_bf16)
    wv = np.asarray(wv, dtype=np.float32).astype(np_bf16)
    wo = np.asarray(wo, dtype=np.float32).astype(np_bf16)

    relg_base, reltail = _host_constants()

    in_maps = []
    for c in range(8):
        b, hf = c // 2, c % 2
        base = hf * (T // 2)
        xsh = np.zeros((TSH, DIM), dtype=np.float32)
        lo = base - HALO
        if lo < 0:
            xsh[HALO:] = x[b, base: base + QTOK]
        else:
            xsh[:] = x[b, lo: base + QTOK]
        xt_c = np.ascontiguousarray(
            xsh.T.reshape(ND, 128, TSH).transpose(1, 0, 2)).astype(np_bf16)
        relg = relg_base.copy()
        if hf == 0:
            relg[:HALO, 0, :, :] = 0         # zero-halo keys are invalid
        in_maps.append({
            "xt": xt_c, "wq": wq, "wk": wk, "wv": wv, "wo": wo,
            "relg": relg, "reltail": reltail,
            "ones_sq": np.ones((128, 128), dtype=np_bf16),
            "ones_row": np.ones((1, 128), dtype=np.float32),
            "ones_col": np.ones((128, 1), dtype=np.float32),
        })

    return in_maps


def kernel(x, wq, wk, wv, wo, q_norm_w, k_norm_w):
    q_norm_w = np.asarray(q_norm_w, dtype=np.float32)
    k_norm_w = np.asarray(k_norm_w, dtype=np.float32)
    qkw = (q_norm_w * k_norm_w).astype(np.float64)
    cval = float(qkw[0]) / math.sqrt(HD)
    assert np.allclose(qkw, qkw[0], rtol=1e-5), (
        "kernel assumes uniform q_norm_w*k_norm_w (score scale folds into "
        "the k-side Sqrt scale/bias)")
    kscale = 1.0 / (DIM * cval * cval)
    keps = EPS / (cval * cval)

    key = ("nc", round(kscale, 9), round(keps, 12))
    if key not in _CACHE:
        _CACHE.clear()
        _CACHE[key] = _build_program(kscale, keps)
    nc = _CACHE[key]
    _CACHE["nc"] = nc
    in_maps = _make_in_maps(x, wq, wk, wv, wo, q_norm_w, k_norm_w)
    _CACHE["in_maps"] = in_maps
    import time as _time
    last_err = None
    for attempt in range(3):
        try:
            res = run_bass_kernel_spmd(nc, in_maps, core_ids=list(range(8)))
            break
        except Exception as e:  # transient NRT/device wedges recover on retry
            last_err = e
            _time.sleep(10 * (attempt + 1))
    else:
        raise last_err

    out = np.empty((B, T, DIM), dtype=np.float32)
    for c in range(8):
        b, hf = c // 2, c % 2
        out[b, hf * QTOK:(hf + 1) * QTOK, :] = res.results[c]["out"]
    return out
